# revision 36
# baseline (speedup 1.0000x reference)
"""MaxMarginLoss Trainium2 kernel (8 NeuronCores, vocab-sharded).

Math (reference):
    out_norm = l2norm(preds^T over D)            [B,S,D]
    voc_norm = l2norm(emb over D)                [V,D]
    tgt      = emb[target]                       [B,S,D]
    d        = out_norm@voc_norm.T - tgt@voc_norm.T
    jmax     = argmax_v d
    loss     = mean_masked(relu(g + cos[jmax] - cos[target]))

Key identity: d = (out_norm - tgt) @ voc_norm.T  -> ONE matmul.  Per-row
positive scaling keeps the argmax, so each device computes
    slab[s,v] = (preds[s] - n_s*tgt[s]) . voc_norm[v]   ( = n_s * d[s,v] )
with NO division on device.  The matmul runs in fp8e4m3 (DoubleRow perf
mode, 2 k-subtiles per instruction) accumulating f32 in PSUM.

Device outputs ONLY per-row block maxes (blocks of W=16 vocab columns).
Per 2048-col PSUM half: DVE reduce_max's the first 128 cols straight from
PSUM; the scalar engine copies the other 1920 cols to SBUF as bf16, which
DVE then folds 16->8->4 with tensor_max (2-byte dtype = 2x_1port DVE
speed) and reduce_max's to block maxes.  This balances Act/DVE/PE at
~1.7us per half.  No DRAM slab, no gathers, no argmax scan on device.
(gpsimd is idle: walrus implements TensorTensor mult but not max on Pool,
and Pool cannot read PSUM.)

Host combine picks the winning (core, block) per row from the 8x256 block
maxes, recomputes that block's 16 exact dots in f64-free numpy (33 MFLOP,
same scale as the cos_target dots it already does), resolves the exact
within-block argmax, and finishes the masked-mean loss.
"""

import os
import sys

import numpy as np

for _p in ("/opt/trn_rl_repo", "/root/.axon_site/_ro/trn_rl_repo"):
    if os.path.isdir(_p) and _p not in sys.path:
        sys.path.insert(0, _p)

import concourse.bass as bass
import concourse.bacc as bacc_mod
import concourse.mybir as mybir
from concourse.tile import TileContext

P = 128
B, S, D, V = 4, 512, 512, 32000
BS = B * S                  # 2048 rows
NCORES = 8
VS = V // NCORES            # 4000 vocab rows per core
VSP = 4096                  # padded vocab per core
KC = D // P                 # 4 k-subtiles of the contraction
NT = BS // P                # 16 row tiles
HALF = 2048                 # columns per PSUM half
W = 16                      # block width for block maxes
NBT = VSP // W              # 256 blocks per row tile
NBH = HALF // W             # 128 blocks per half
NVALID = VS // W            # 250 non-pad blocks per core
DCOLS = 512                 # per-half cols DVE reduces straight from PSUM
DNB = DCOLS // W            # 32
ACOLS = HALF - DCOLS        # 1536 cols Act copies to bf16 for DVE folds
ANB = ACOLS // W            # 96
SCALE_E = 0.125
SCALE_V = 16.0
GAMMA = 0.5

F32 = mybir.dt.float32
BF16 = mybir.dt.bfloat16
F8 = mybir.dt.float8e4

_CACHED = {}


def build_nc():
    nc = bacc_mod.Bacc()

    eT8 = nc.declare_dram_parameter("eT8", [P, KC * BS], F8, isOutput=False)
    voc8 = nc.declare_dram_parameter("voc8", [P, KC * VSP], F8, isOutput=False)
    o_bm = nc.declare_dram_parameter(
        "o_bm", [P, 2 * NT * ANB], BF16, isOutput=True)   # [h, t, 84] tree
    o_bmd = nc.declare_dram_parameter(
        "o_bmd", [P, 2 * NT * DNB], F32, isOutput=True)   # [h, t, 44] direct

    with TileContext(nc) as tc:
        with (
            tc.tile_pool(name="const", bufs=1) as cpool,
            tc.tile_pool(name="bmp", bufs=3) as bmp,
            tc.tile_pool(name="psp", bufs=2, space="PSUM") as psp,
        ):
            # Separate tiles per input chunk: tile-granular DMA
            # semaphores mean a matmul only waits for ITS chunk's DMA,
            # not for the whole input load.
            vC = [cpool.tile([P, KC * 512], F8, tag=f"vC{j}", name=f"vC{j}")
                  for j in range(8)]
            eP0 = cpool.tile([P, 512], F8, tag="eP0")
            eP1 = cpool.tile([P, 1536], F8, tag="eP1")
            eP2 = cpool.tile([P, KC * BS - 2048], F8, tag="eP2")

            # PE warm-up burst (p-state ramp): small scratch, short
            # 128-row matmuls, so PE gets busy ~6.5us while inputs fly.
            wU = cpool.tile([P, 512], F8, tag="wU")
            nc.vector.memset(wU, 0.0)
            psw = psp.tile([P, ACOLS], F32, tag="psB", name="ps_warm")
            for i in range(22):
                nc.tensor.matmul(
                    psw[:, :128],
                    lhsT=wU[:, 0:256].rearrange("p (k m) -> p k m", k=2),
                    rhs=wU[:, 256:512].rearrange("p (k m) -> p k m", k=2),
                    start=True, stop=True,
                    perf_mode=mybir.MatmulPerfMode.DoubleRow,
                )

            def vchunk(j):
                return slice(j * KC * 512, (j + 1) * KC * 512)

            # sync/scalar interleave the first 4 voc chunks (needed by the
            # first row tile) between the eT pieces; gpsimd (slow SWDGE
            # setup) takes late chunks.
            nc.sync.dma_start(eP0, eT8[:, 0:512])
            nc.scalar.dma_start(vC[0], voc8[:, vchunk(0)])
            nc.sync.dma_start(vC[1], voc8[:, vchunk(1)])
            nc.scalar.dma_start(vC[2], voc8[:, vchunk(2)])
            nc.sync.dma_start(vC[3], voc8[:, vchunk(3)])
            nc.scalar.dma_start(eP1, eT8[:, 512:2048])
            nc.sync.dma_start(eP2, eT8[:, 2048:KC * BS])
            nc.gpsimd.dma_start(vC[4], voc8[:, vchunk(4)])
            nc.gpsimd.dma_start(vC[5], voc8[:, vchunk(5)])
            nc.gpsimd.dma_start(vC[6], voc8[:, vchunk(6)])
            nc.gpsimd.dma_start(vC[7], voc8[:, vchunk(7)])

            # lhsT views per eT piece: [p, t_local, k, m] (256B runs)
            eV = [eP0[:].rearrange("p (t k m) -> p t k m", t=1, k=KC),
                  eP1[:].rearrange("p (t k m) -> p t k m", t=3, k=KC),
                  eP2[:].rearrange("p (t k m) -> p t k m", t=NT - 4, k=KC)]

            def lhsT_view(t, q):
                piece = 0 if t == 0 else (1 if t < 4 else 2)
                tl = t - (0, 1, 4)[piece]
                return eV[piece][:, tl, 2 * q:2 * q + 2, :]

            # rhs views per voc chunk: [p, k, m] (1KB runs)
            vV = [vC[j][:].rearrange("p (k m) -> p k m", k=KC)
                  for j in range(8)]

            # Phase-major: all row tiles of half 0 first (needs only voc
            # chunks 0-3, so compute starts before chunks 4-7 land).
            bmq = bdq = None
            for h in range(2):
                for t in range(NT):
                    if t % 4 == 0:
                        bmq = bmp.tile([P, 4 * ANB], BF16, tag="bmq")
                        bdq = bmp.tile([P, 4 * DNB], F32, tag="bdq")
                    g = t % 4
                    # Two PSUM tiles per half: DVE reduces psA while Act
                    # copies psB -- same-tile readers would be serialized
                    # by the tile framework, separate tiles overlap.
                    psA = psp.tile([P, DCOLS], F32, tag="psA")
                    psB = psp.tile([P, ACOLS], F32, tag="psB")
                    for c in range(4):
                        out = psA[:, 0:512] if c == 0 else \
                            psB[:, (c - 1) * 512:c * 512]
                        for q in range(2):
                            nc.tensor.matmul(
                                out,
                                lhsT=lhsT_view(t, q),
                                rhs=vV[h * 4 + c][:, 2 * q:2 * q + 2, :],
                                start=(q == 0), stop=(q == 1),
                                perf_mode=mybir.MatmulPerfMode.DoubleRow,
                            )
                    # Readout: DVE direct-reduces the first 704 cols from
                    # PSUM (DVE can read only ONE PSUM operand, 1 elem/cyc
                    # f32); Act bf16-copies the other 1344 cols, which DVE
                    # then folds 16->1 at bf16 2x_1port speed.
                    nc.vector.reduce_max(
                        bdq[:, g * DNB:(g + 1) * DNB],
                        psA[:].rearrange("p (b w) -> p b w", w=W),
                        axis=mybir.AxisListType.X,
                    )
                    stg = bmp.tile([P, ACOLS], BF16, tag="stg")
                    nc.scalar.copy(stg, psB)
                    sv = stg[:].rearrange("p (b w) -> p b w", w=W)
                    fs = bmp.tile([P, ANB * 14], BF16, tag="fs")
                    f1r = fs[:, 0:ANB * 8].rearrange("p (b w) -> p b w", w=8)
                    f2r = fs[:, ANB * 8:ANB * 12].rearrange(
                        "p (b w) -> p b w", w=4)
                    f3r = fs[:, ANB * 12:ANB * 14].rearrange(
                        "p (b w) -> p b w", w=2)
                    nc.vector.tensor_max(f1r, sv[:, :, 0:8], sv[:, :, 8:16])
                    nc.vector.tensor_max(f2r, f1r[:, :, 0:4], f1r[:, :, 4:8])
                    nc.vector.tensor_max(f3r, f2r[:, :, 0:2], f2r[:, :, 2:4])
                    nc.vector.tensor_max(
                        bmq[:, g * ANB:(g + 1) * ANB].rearrange(
                            "p (b o) -> p b o", o=1),
                        f3r[:, :, 0:1], f3r[:, :, 1:2])
                    # ship 4 tiles' block maxes at a time
                    if t % 4 == 3:
                        base = h * NT + (t - 3)
                        nc.sync.dma_start(
                            o_bm[:, base * ANB:(base + 4) * ANB], bmq)
                        nc.sync.dma_start(
                            o_bmd[:, base * DNB:(base + 4) * DNB], bdq)

    return nc


def get_nc():
    if "nc" not in _CACHED:
        _CACHED["nc"] = build_nc()
    return _CACHED["nc"]


def _prep(preds, emb_weight, target):
    preds = np.ascontiguousarray(np.asarray(preds, dtype=np.float32))     # [B,D,S]
    emb = np.ascontiguousarray(np.asarray(emb_weight, dtype=np.float32))  # [V,D]
    tgt_idx = np.asarray(target).astype(np.int64).reshape(-1)             # [BS]

    predsN = np.ascontiguousarray(preds.transpose(0, 2, 1).reshape(BS, D))
    n = np.maximum(np.sqrt((predsN ** 2).sum(axis=1)), 1e-12).astype(np.float32)
    tgtN = emb[tgt_idx]                                                   # [BS,D]
    er = predsN - n[:, None] * tgtN                                       # [BS,D]
    vocn = emb / np.maximum(
        np.sqrt((emb ** 2).sum(axis=1, keepdims=True)), 1e-12)            # [V,D]
    return predsN, n, tgtN, er, vocn, tgt_idx


def make_in_maps(preds, emb_weight, target):
    import ml_dtypes
    _, _, _, er, vocn, _ = _prep(preds, emb_weight, target)

    e8 = ((er.T) * SCALE_E).astype(ml_dtypes.float8_e4m3)                 # [D,BS]
    # [p, t, k, m]: row-tile-major so per-tile DMA slices are contiguous
    eT8 = np.ascontiguousarray(
        e8.reshape(KC, P, NT, P).transpose(1, 2, 0, 3).reshape(P, KC * BS))

    in_maps = []
    for c in range(NCORES):
        sh = np.zeros((VSP, D), np.float32)
        sh[:VS] = vocn[c * VS:(c + 1) * VS]
        v8 = (sh.T * SCALE_V).astype(ml_dtypes.float8_e4m3)               # [D,VSP]
        # [p, j, k, m]: chunk-major so per-chunk DMA slices are contiguous
        voc8 = np.ascontiguousarray(
            v8.reshape(KC, P, 8, 512).transpose(1, 2, 0, 3).reshape(P, KC * VSP))
        in_maps.append({"eT8": eT8, "voc8": voc8})
    return in_maps


def combine(results, preds, emb_weight, target, pad_id):
    predsN, n, tgtN, er, vocn, tgt_idx = _prep(preds, emb_weight, target)

    # o_bmd [P, h, t, 44] f32 covers cols [0,704); o_bm [P, h, t, 84] bf16
    # covers cols [704,2048) of each half.  M[row, core*NBT + h*NBH + b],
    # row j = t*128 + p; block g = h*NBH + b covers shard cols [16g, 16g+16)
    bd = np.stack([np.asarray(r["o_bmd"]) for r in results]).reshape(
        NCORES, P, 2, NT, DNB)
    bt = np.stack([np.asarray(r["o_bm"]).astype(np.float32)
                   for r in results]).reshape(NCORES, P, 2, NT, ANB)
    M = np.concatenate([bd, bt], axis=4).transpose(3, 1, 0, 2, 4).reshape(
        BS, NCORES * NBT)
    pad_mask = np.tile(np.arange(NBT) >= NVALID, NCORES)
    M[:, pad_mask] = -np.inf

    win = np.argmax(M, axis=1)
    core, blk = win // NBT, win % NBT
    cand = core[:, None] * VS + blk[:, None] * W + np.arange(W)[None, :]  # [BS,W]

    dblk = np.einsum('rd,rwd->rw', er, vocn[cand])
    k = np.argmax(dblk, axis=1)
    jmax = cand[np.arange(BS), k]

    cosmax = (predsN * vocn[jmax]).sum(axis=1) / n
    costgt = (predsN * tgtN).sum(axis=1) / (
        np.maximum(np.sqrt((tgtN ** 2).sum(axis=1)), 1e-12) * n)
    diff = np.maximum(np.float32(GAMMA) + cosmax - costgt, 0.0).astype(np.float32)
    mask = tgt_idx != int(np.asarray(pad_id))
    denom = np.float32(mask.sum())
    loss = np.float32(np.where(mask, diff, np.float32(0.0)).sum() / denom)
    return np.asarray(loss, dtype=np.float32)


def run_cores(in_maps, trace=False):
    from concourse.bass_utils import run_bass_kernel_spmd
    nc = get_nc()
    if not nc.is_finalized():
        nc.finalize()
    return run_bass_kernel_spmd(nc, in_maps, list(range(NCORES)), trace=trace)


def kernel(preds, emb_weight, target, pad_id):
    in_maps = make_in_maps(preds, emb_weight, target)
    res = run_cores(in_maps, trace=False)
    return combine(res.results, preds, emb_weight, target, pad_id)


# revision 37
# speedup vs baseline: 1.0185x; 1.0185x over previous
"""MaxMarginLoss Trainium2 kernel (8 NeuronCores, vocab-sharded).

Math (reference):
    out_norm = l2norm(preds^T over D)            [B,S,D]
    voc_norm = l2norm(emb over D)                [V,D]
    tgt      = emb[target]                       [B,S,D]
    d        = out_norm@voc_norm.T - tgt@voc_norm.T
    jmax     = argmax_v d
    loss     = mean_masked(relu(g + cos[jmax] - cos[target]))

Key identity: d = (out_norm - tgt) @ voc_norm.T  -> ONE matmul.  Per-row
positive scaling keeps the argmax, so each device computes
    slab[s,v] = (preds[s] - n_s*tgt[s]) . voc_norm[v]   ( = n_s * d[s,v] )
with NO division on device.  The matmul runs in fp8e4m3 (DoubleRow perf
mode, 2 k-subtiles per instruction) accumulating f32 in PSUM.

Device outputs ONLY per-row block maxes (blocks of W=16 vocab columns).
Per 2048-col PSUM half: DVE reduce_max's the first 128 cols straight from
PSUM; the scalar engine copies the other 1920 cols to SBUF as bf16, which
DVE then folds 16->8->4 with tensor_max (2-byte dtype = 2x_1port DVE
speed) and reduce_max's to block maxes.  This balances Act/DVE/PE at
~1.7us per half.  No DRAM slab, no gathers, no argmax scan on device.
(gpsimd is idle: walrus implements TensorTensor mult but not max on Pool,
and Pool cannot read PSUM.)

Host combine picks the winning (core, block) per row from the 8x256 block
maxes, recomputes that block's 16 exact dots in f64-free numpy (33 MFLOP,
same scale as the cos_target dots it already does), resolves the exact
within-block argmax, and finishes the masked-mean loss.
"""

import os
import sys

import numpy as np

for _p in ("/opt/trn_rl_repo", "/root/.axon_site/_ro/trn_rl_repo"):
    if os.path.isdir(_p) and _p not in sys.path:
        sys.path.insert(0, _p)

import concourse.bass as bass
import concourse.bacc as bacc_mod
import concourse.mybir as mybir
from concourse.tile import TileContext

P = 128
B, S, D, V = 4, 512, 512, 32000
BS = B * S                  # 2048 rows
NCORES = 8
VS = V // NCORES            # 4000 vocab rows per core
VSP = 4096                  # padded vocab per core
KC = D // P                 # 4 k-subtiles of the contraction
NT = BS // P                # 16 row tiles
HALF = 2048                 # columns per PSUM half
W = 16                      # block width for block maxes
NBT = VSP // W              # 256 blocks per row tile
NBH = HALF // W             # 128 blocks per half
NVALID = VS // W            # 250 non-pad blocks per core
DCOLS = 512                 # per-half cols DVE reduces straight from PSUM
DNB = DCOLS // W            # 32
ACOLS = HALF - DCOLS        # 1536 cols Act copies to bf16 for DVE folds
ANB = ACOLS // W            # 96
SCALE_E = 0.125
SCALE_V = 16.0
GAMMA = 0.5

F32 = mybir.dt.float32
BF16 = mybir.dt.bfloat16
F8 = mybir.dt.float8e4

_CACHED = {}


def build_nc():
    nc = bacc_mod.Bacc()

    eT8 = nc.declare_dram_parameter("eT8", [P, KC * BS], F8, isOutput=False)
    voc8 = nc.declare_dram_parameter("voc8", [P, KC * VSP], F8, isOutput=False)
    o_bm = nc.declare_dram_parameter(
        "o_bm", [P, 2 * NT * ANB], BF16, isOutput=True)   # [h, t, 84] tree
    o_bmd = nc.declare_dram_parameter(
        "o_bmd", [P, 2 * NT * DNB], F32, isOutput=True)   # [h, t, 44] direct

    with TileContext(nc) as tc:
        with (
            tc.tile_pool(name="const", bufs=1) as cpool,
            tc.tile_pool(name="bmp", bufs=3) as bmp,
            tc.tile_pool(name="psp", bufs=2, space="PSUM") as psp,
        ):
            # Separate tiles per input chunk: tile-granular DMA
            # semaphores mean a matmul only waits for ITS chunk's DMA,
            # not for the whole input load.
            vC = [cpool.tile([P, KC * 512], F8, tag=f"vC{j}", name=f"vC{j}")
                  for j in range(8)]
            eP0 = cpool.tile([P, 512], F8, tag="eP0")
            eP1 = cpool.tile([P, 1536], F8, tag="eP1")
            eP2 = cpool.tile([P, KC * BS - 2048], F8, tag="eP2")

            # PE warm-up burst (p-state ramp): small scratch, short
            # 128-row matmuls, so PE gets busy ~6.5us while inputs fly.
            wU = cpool.tile([P, 512], F8, tag="wU")
            nc.vector.memset(wU, 0.0)
            psw = psp.tile([P, ACOLS], F32, tag="psB", name="ps_warm")
            for i in range(18):
                nc.tensor.matmul(
                    psw[:, :128],
                    lhsT=wU[:, 0:256].rearrange("p (k m) -> p k m", k=2),
                    rhs=wU[:, 256:512].rearrange("p (k m) -> p k m", k=2),
                    start=True, stop=True,
                    perf_mode=mybir.MatmulPerfMode.DoubleRow,
                )

            def vchunk(j):
                return slice(j * KC * 512, (j + 1) * KC * 512)

            # sync/scalar interleave the first 4 voc chunks (needed by the
            # first row tile) between the eT pieces; gpsimd (slow SWDGE
            # setup) takes late chunks.
            nc.sync.dma_start(eP0, eT8[:, 0:512])
            nc.scalar.dma_start(vC[0], voc8[:, vchunk(0)])
            nc.sync.dma_start(vC[1], voc8[:, vchunk(1)])
            nc.scalar.dma_start(vC[2], voc8[:, vchunk(2)])
            nc.sync.dma_start(vC[3], voc8[:, vchunk(3)])
            nc.scalar.dma_start(eP1, eT8[:, 512:2048])
            nc.sync.dma_start(eP2, eT8[:, 2048:KC * BS])
            nc.gpsimd.dma_start(vC[4], voc8[:, vchunk(4)])
            nc.gpsimd.dma_start(vC[5], voc8[:, vchunk(5)])
            nc.gpsimd.dma_start(vC[6], voc8[:, vchunk(6)])
            nc.gpsimd.dma_start(vC[7], voc8[:, vchunk(7)])

            # lhsT views per eT piece: [p, t_local, k, m] (256B runs)
            eV = [eP0[:].rearrange("p (t k m) -> p t k m", t=1, k=KC),
                  eP1[:].rearrange("p (t k m) -> p t k m", t=3, k=KC),
                  eP2[:].rearrange("p (t k m) -> p t k m", t=NT - 4, k=KC)]

            def lhsT_view(t, q):
                piece = 0 if t == 0 else (1 if t < 4 else 2)
                tl = t - (0, 1, 4)[piece]
                return eV[piece][:, tl, 2 * q:2 * q + 2, :]

            # rhs views per voc chunk: [p, k, m] (1KB runs)
            vV = [vC[j][:].rearrange("p (k m) -> p k m", k=KC)
                  for j in range(8)]

            # Phase-major: all row tiles of half 0 first (needs only voc
            # chunks 0-3, so compute starts before chunks 4-7 land).
            bmq = bdq = None
            for h in range(2):
                for t in range(NT):
                    if t % 4 == 0:
                        bmq = bmp.tile([P, 4 * ANB], BF16, tag="bmq")
                        bdq = bmp.tile([P, 4 * DNB], F32, tag="bdq")
                    g = t % 4
                    # Two PSUM tiles per half: DVE reduces psA while Act
                    # copies psB -- same-tile readers would be serialized
                    # by the tile framework, separate tiles overlap.
                    psA = psp.tile([P, DCOLS], F32, tag="psA")
                    psB = psp.tile([P, ACOLS], F32, tag="psB")
                    for c in range(4):
                        out = psA[:, 0:512] if c == 0 else \
                            psB[:, (c - 1) * 512:c * 512]
                        for q in range(2):
                            nc.tensor.matmul(
                                out,
                                lhsT=lhsT_view(t, q),
                                rhs=vV[h * 4 + c][:, 2 * q:2 * q + 2, :],
                                start=(q == 0), stop=(q == 1),
                                perf_mode=mybir.MatmulPerfMode.DoubleRow,
                            )
                    # Readout: DVE direct-reduces the first 704 cols from
                    # PSUM (DVE can read only ONE PSUM operand, 1 elem/cyc
                    # f32); Act bf16-copies the other 1344 cols, which DVE
                    # then folds 16->1 at bf16 2x_1port speed.
                    nc.vector.reduce_max(
                        bdq[:, g * DNB:(g + 1) * DNB],
                        psA[:].rearrange("p (b w) -> p b w", w=W),
                        axis=mybir.AxisListType.X,
                    )
                    stg = bmp.tile([P, ACOLS], BF16, tag="stg")
                    nc.scalar.copy(stg, psB)
                    sv = stg[:].rearrange("p (b w) -> p b w", w=W)
                    fs = bmp.tile([P, ANB * 14], BF16, tag="fs")
                    f1r = fs[:, 0:ANB * 8].rearrange("p (b w) -> p b w", w=8)
                    f2r = fs[:, ANB * 8:ANB * 12].rearrange(
                        "p (b w) -> p b w", w=4)
                    f3r = fs[:, ANB * 12:ANB * 14].rearrange(
                        "p (b w) -> p b w", w=2)
                    nc.vector.tensor_max(f1r, sv[:, :, 0:8], sv[:, :, 8:16])
                    nc.vector.tensor_max(f2r, f1r[:, :, 0:4], f1r[:, :, 4:8])
                    nc.vector.tensor_max(f3r, f2r[:, :, 0:2], f2r[:, :, 2:4])
                    nc.vector.tensor_max(
                        bmq[:, g * ANB:(g + 1) * ANB].rearrange(
                            "p (b o) -> p b o", o=1),
                        f3r[:, :, 0:1], f3r[:, :, 1:2])
                    # ship 4 tiles' block maxes at a time
                    if t % 4 == 3:
                        base = h * NT + (t - 3)
                        nc.sync.dma_start(
                            o_bm[:, base * ANB:(base + 4) * ANB], bmq)
                        nc.sync.dma_start(
                            o_bmd[:, base * DNB:(base + 4) * DNB], bdq)

    return nc


def get_nc():
    if "nc" not in _CACHED:
        _CACHED["nc"] = build_nc()
    return _CACHED["nc"]


def _prep(preds, emb_weight, target):
    preds = np.ascontiguousarray(np.asarray(preds, dtype=np.float32))     # [B,D,S]
    emb = np.ascontiguousarray(np.asarray(emb_weight, dtype=np.float32))  # [V,D]
    tgt_idx = np.asarray(target).astype(np.int64).reshape(-1)             # [BS]

    predsN = np.ascontiguousarray(preds.transpose(0, 2, 1).reshape(BS, D))
    n = np.maximum(np.sqrt((predsN ** 2).sum(axis=1)), 1e-12).astype(np.float32)
    tgtN = emb[tgt_idx]                                                   # [BS,D]
    er = predsN - n[:, None] * tgtN                                       # [BS,D]
    vocn = emb / np.maximum(
        np.sqrt((emb ** 2).sum(axis=1, keepdims=True)), 1e-12)            # [V,D]
    return predsN, n, tgtN, er, vocn, tgt_idx


def make_in_maps(preds, emb_weight, target):
    import ml_dtypes
    _, _, _, er, vocn, _ = _prep(preds, emb_weight, target)

    e8 = ((er.T) * SCALE_E).astype(ml_dtypes.float8_e4m3)                 # [D,BS]
    # [p, t, k, m]: row-tile-major so per-tile DMA slices are contiguous
    eT8 = np.ascontiguousarray(
        e8.reshape(KC, P, NT, P).transpose(1, 2, 0, 3).reshape(P, KC * BS))

    in_maps = []
    for c in range(NCORES):
        sh = np.zeros((VSP, D), np.float32)
        sh[:VS] = vocn[c * VS:(c + 1) * VS]
        v8 = (sh.T * SCALE_V).astype(ml_dtypes.float8_e4m3)               # [D,VSP]
        # [p, j, k, m]: chunk-major so per-chunk DMA slices are contiguous
        voc8 = np.ascontiguousarray(
            v8.reshape(KC, P, 8, 512).transpose(1, 2, 0, 3).reshape(P, KC * VSP))
        in_maps.append({"eT8": eT8, "voc8": voc8})
    return in_maps


def combine(results, preds, emb_weight, target, pad_id):
    predsN, n, tgtN, er, vocn, tgt_idx = _prep(preds, emb_weight, target)

    # o_bmd [P, h, t, 44] f32 covers cols [0,704); o_bm [P, h, t, 84] bf16
    # covers cols [704,2048) of each half.  M[row, core*NBT + h*NBH + b],
    # row j = t*128 + p; block g = h*NBH + b covers shard cols [16g, 16g+16)
    bd = np.stack([np.asarray(r["o_bmd"]) for r in results]).reshape(
        NCORES, P, 2, NT, DNB)
    bt = np.stack([np.asarray(r["o_bm"]).astype(np.float32)
                   for r in results]).reshape(NCORES, P, 2, NT, ANB)
    M = np.concatenate([bd, bt], axis=4).transpose(3, 1, 0, 2, 4).reshape(
        BS, NCORES * NBT)
    pad_mask = np.tile(np.arange(NBT) >= NVALID, NCORES)
    M[:, pad_mask] = -np.inf

    win = np.argmax(M, axis=1)
    core, blk = win // NBT, win % NBT
    cand = core[:, None] * VS + blk[:, None] * W + np.arange(W)[None, :]  # [BS,W]

    dblk = np.einsum('rd,rwd->rw', er, vocn[cand])
    k = np.argmax(dblk, axis=1)
    jmax = cand[np.arange(BS), k]

    cosmax = (predsN * vocn[jmax]).sum(axis=1) / n
    costgt = (predsN * tgtN).sum(axis=1) / (
        np.maximum(np.sqrt((tgtN ** 2).sum(axis=1)), 1e-12) * n)
    diff = np.maximum(np.float32(GAMMA) + cosmax - costgt, 0.0).astype(np.float32)
    mask = tgt_idx != int(np.asarray(pad_id))
    denom = np.float32(mask.sum())
    loss = np.float32(np.where(mask, diff, np.float32(0.0)).sum() / denom)
    return np.asarray(loss, dtype=np.float32)


def run_cores(in_maps, trace=False):
    from concourse.bass_utils import run_bass_kernel_spmd
    nc = get_nc()
    if not nc.is_finalized():
        nc.finalize()
    return run_bass_kernel_spmd(nc, in_maps, list(range(NCORES)), trace=trace)


def kernel(preds, emb_weight, target, pad_id):
    in_maps = make_in_maps(preds, emb_weight, target)
    res = run_cores(in_maps, trace=False)
    return combine(res.results, preds, emb_weight, target, pad_id)
